# revision 32
# baseline (speedup 1.0000x reference)
"""AFNO-style spectral MLP (nn_AFMM_57166014709855) as a Bass/Tile kernel on 8 TRN2 cores.

Math: y = irfft2( (cMLP(rfft2(x)) * rfft2(x)) ) with a grouped complex MLP in
the frequency domain. All FFTs are implemented as DFT matmuls on the tensor
engine. Data-parallel over batch: core b handles x[b] (512,128,128); weights
replicated; no collectives.

Per-core dataflow for each group g (d=128 channels):
  S1 (contract h): per ch:  psum[w,256] = x_c^T @ [FHr|FHi]        (f32r)
  S2 (contract w): per kh:  psum[c,130] = Yr@[Fwr|Fwi] + Yi@[-Fwi|Fwr]  (bf16)
  S3 (contract c): chunks:  H = relu(W1 (x) S + b1)                 (bf16)
  S4 (contract c): chunks:  O = W2 (x) H + b2; gate G = O * S       (bf16)
  S5: PE-transpose G -> GT [kh, (kw,c)]                             (bf16)
  S6 (contract kh): per ch: psum[kw,256] = GTr@[IHr|IHi] + GTi@[-IHi|IHr]
  S7 (contract kw): per ch: y_c[h,w] = Vr^T@ICr + Vi^T@ICi
"""

import numpy as np

import concourse.bass as bass
import concourse.tile as tile
import concourse.mybir as mybir
from concourse import bacc
from concourse.bass_utils import run_bass_kernel_spmd

F32 = mybir.dt.float32
F32R = mybir.dt.float32r
BF16 = mybir.dt.bfloat16
AF = mybir.ActivationFunctionType
ALU = mybir.AluOpType

B, C, H, W = 8, 512, 128, 128
G, D, KH, KW = 4, 128, 128, 65
NCORES = 8


def _consts():
    s = 1.0 / np.sqrt(128.0)
    k = np.arange(KW)
    n = np.arange(128)
    th_w = np.outer(n, k) * (2 * np.pi / W)          # [w, kw]
    Fwr = np.cos(th_w) * s
    Fwi = -np.sin(th_w) * s
    th_h = np.outer(n, n) * (2 * np.pi / H)          # [h, kh] (symmetric)
    FHr_T = np.cos(th_h) * s
    FHi_T = -np.sin(th_h) * s
    IHr_T = np.cos(th_h) * s
    IHi_T = np.sin(th_h) * s
    a = np.full(KW, 2.0)
    a[0] = 1.0
    a[64] = 1.0
    th_c = np.outer(k, n) * (2 * np.pi / W)          # [kw, w]
    ICr = a[:, None] * np.cos(th_c) * s
    ICi = -a[:, None] * np.sin(th_c) * s
    f32 = lambda x: np.ascontiguousarray(x, dtype=np.float32)
    return {
        "FHRI": f32(np.concatenate([FHr_T[:, 0:65], FHi_T[:, 0:65]], axis=1)),  # [128, 130]
        "FW_A": f32(np.concatenate([Fwr, Fwi], axis=1)),       # [128, 130]
        "FW_B": f32(np.concatenate([-Fwi, Fwr], axis=1)),      # [128, 130]
        "FW_C": f32(np.concatenate([Fwi, -Fwr], axis=1)),      # [128, 130]
        "IH_A": f32(np.concatenate([IHr_T, IHi_T], axis=1)),   # [128, 256]
        "IH_B": f32(np.concatenate([-IHi_T, IHr_T], axis=1)),  # [128, 256]
        "ICr": f32(ICr),                                       # [65, 128]
        "ICi": f32(ICi),                                       # [65, 128]
        "IDENT": f32(np.eye(128)),                             # [128, 128]
    }


def _chunks(total, step):
    off = 0
    while off < total:
        n = min(step, total - off)
        yield off, n
        off += n


def build_nc():
    nc = bacc.Bacc(None, target_bir_lowering=False, debug=False)

    x_ext = nc.declare_dram_parameter("x", [C, H, W], F32, isOutput=False)
    w1_ext = nc.declare_dram_parameter("w1", [2, G, D, D], F32, isOutput=False)
    w2_ext = nc.declare_dram_parameter("w2", [2, G, D, D], F32, isOutput=False)
    b1_ext = nc.declare_dram_parameter("b1", [2, G, D], F32, isOutput=False)
    b2_ext = nc.declare_dram_parameter("b2", [2, G, D], F32, isOutput=False)
    out_ext = nc.declare_dram_parameter("out", [C, H, W], F32, isOutput=True)

    cdat = _consts()
    cdram = {k: nc.inline_tensor(v, f"c_{k}") for k, v in cdat.items()}

    with tile.TileContext(nc) as tc:
        with (
            tc.tile_pool(name="cst", bufs=1) as cst,
            tc.tile_pool(name="big", bufs=1) as big,
            tc.tile_pool(name="work", bufs=1) as work,
            tc.tile_pool(name="psum", bufs=8, space="PSUM") as psp,
        ):
            # ---- constants: DMA fp32 staging, convert once on gpsimd ----
            def staged(k):
                t = cst.tile([128, 256], F32, tag="stg", bufs=2, name=f"stg_{k}")
                v = t[:, 0:cdat[k].shape[1]][0:cdat[k].shape[0], :]
                nc.scalar.dma_start(v, cdram[k][:])
                return v
            FHRI = cst.tile([128, 130], BF16, name="FHRI")
            nc.vector.tensor_copy(FHRI[:], staged("FHRI"))
            cbf = {}
            for k in ["FW_A", "FW_B", "FW_C", "IH_A", "IH_B", "ICr", "ICi", "IDENT"]:
                t = cst.tile(list(cdat[k].shape), BF16, name=f"bf_{k}")
                nc.vector.tensor_copy(t[:], staged(k))
                cbf[k] = t

            # ---- weights/biases for all 4 groups ----
            wts = []  # per group dict
            for g in range(G):
                d = {}
                for nm, ext, comp in [
                    ("w1r", w1_ext, 0), ("w1i", w1_ext, 1),
                    ("w2r", w2_ext, 0), ("w2i", w2_ext, 1),
                ]:
                    wstg = cst.tile([128, 128], F32, tag="wstg", bufs=2,
                                    name=f"wstg_{g}_{nm}")
                    nc.scalar.dma_start(wstg[:], ext[comp, g])
                    wbf = cst.tile([128, 128], BF16, name=f"w_{g}_{nm}")
                    nc.vector.tensor_copy(wbf[:], wstg[:])
                    d[nm] = wbf
                    if nm in ("w1i", "w2i"):
                        wneg = cst.tile([128, 128], BF16, name=f"w_{g}_{nm}n")
                        nc.vector.tensor_scalar_mul(wneg[:], wstg[:], -1.0)
                        d[nm + "n"] = wneg
                for nm, ext, comp in [
                    ("b1r", b1_ext, 0), ("b1i", b1_ext, 1),
                    ("b2r", b2_ext, 0), ("b2i", b2_ext, 1),
                ]:
                    bt = cst.tile([128, 1], F32, name=f"b_{g}_{nm}")
                    nc.scalar.dma_start(bt[:], ext[comp, g].rearrange("(p one) -> p one", one=1))
                    d[nm] = bt
                wts.append(d)

            # ---- per-group pipeline ----
            PS_SHAPE = [128, 1024]  # 2 PSUM banks
            NPRE = 16

            def emit_xchunk(rep, g, ch):
                c0 = g * D
                xf = work.tile([128, 1024], F32, tag="xf", bufs=3,
                               name=f"xf_{rep}_{g}_{ch}", uniquify=False)
                eng = nc.sync if (ch // 8) % 2 == 0 else nc.scalar
                eng.dma_start(
                    xf[:].rearrange("p (c w) -> p c w", w=128),
                    x_ext[c0 + ch:c0 + ch + 8].transpose([1, 0, 2]))
                xr = work.tile([128, 1024], BF16, tag="xr", bufs=4,
                               name=f"xr_{rep}_{g}_{ch}", uniquify=False)
                nc.gpsimd.tensor_copy(xr[:, 0:1024], xf[:, 0:1024])
                nc.vector.tensor_copy(xr[:, 1024:2048], xf[:, 1024:2048])
                return xr

            for rep in range(repeat):
              xr_pre = [[] for _ in range(G)]
              for ch in range(0, 16 * NPRE, 16):
                  xr_pre[0].append(emit_xchunk(rep, 0, ch, wide=(rep == 0)))
              for g in range(G):
                wt = wts[g]
                c0 = g * D

                # x load + convert to bf16, 8 channels per chunk
                # (first NPRE chunks of each group are emitted early, after the
                #  previous group's S2, so converts overlap the gate/MLP era)
                xr_tiles = xr_pre[g]
                for ch in range(16 * len(xr_tiles), D, 16):
                    xr_tiles.append(emit_xchunk(rep, g, ch, wide=(rep == 0 and g == 0)))

                # S1: forward H-FFT (hermitian half: kh 0..64)
                # -> Y [w, c*130 + comp*65 + kh] bf16; 6 channels per psum tile
                Y = big.tile([128, D * 130], BF16, tag="Y", bufs=1, name=f"Y_{rep}_{g}")
                for ti, (ch0, n) in enumerate(_chunks(D, 6)):
                    ps = psp.tile(PS_SHAPE, F32, tag="ps", bufs=4, name=f"s1p_{rep}_{g}_{ch0}")
                    for j in range(n):
                        c = ch0 + j
                        off = (j % 3) * 130 + (j // 3) * 512
                        lhsT = xr_tiles[c // 16][:, (c % 16) * 128:(c % 16) * 128 + 128]
                        nc.tensor.matmul(ps[:, off:off + 130], lhsT, FHRI[:],
                                         start=True, stop=True)
                    if n == 6:
                        srcv = ps[:].rearrange("p (b x) -> p b x", b=2)[:, :, 0:390]
                        dv = Y[:, ch0 * 130:(ch0 + 6) * 130].rearrange(
                            "p (b x) -> p b x", b=2)
                    else:
                        srcv = ps[:, 0:n * 130]
                        dv = Y[:, ch0 * 130:(ch0 + n) * 130]
                    if ti % 2 == 0:
                        nc.scalar.copy(dv, srcv)
                    else:
                        nc.vector.tensor_copy(dv, srcv)

                # S2: forward W-FFT -> Sr/Si [c, kh*65 + kw] bf16; 6 kh per tile
                Sr = big.tile([128, KH * KW], BF16, tag="Sr", bufs=1, name=f"Sr_{rep}_{g}")
                Si = big.tile([128, KH * KW], BF16, tag="Si", bufs=1, name=f"Si_{rep}_{g}")
                Yv = Y[:].rearrange("p (c t k) -> p c t k", t=2, k=65)
                for ti, (kh0, n) in enumerate(_chunks(KH, 6)):
                    ps = psp.tile(PS_SHAPE, F32, tag="ps", bufs=4, name=f"s2p_{rep}_{g}_{kh0}")
                    for j in range(n):
                        kh = kh0 + j
                        idx = kh if kh <= 64 else 128 - kh
                        rhsB = cbf["FW_B"] if kh <= 64 else cbf["FW_C"]
                        off = j * 130 if j < 3 else 512 + (j - 3) * 130
                        o = ps[:, off:off + 130]
                        nc.tensor.matmul(o, Yv[:, :, 0, idx], cbf["FW_A"][:],
                                         start=True, stop=False)
                        nc.tensor.matmul(o, Yv[:, :, 1, idx], rhsB[:],
                                         start=False, stop=True)
                    for t, dstT in ((0, Sr), (1, Si)):
                        eng_scalar = (ti + t) % 2 == 0
                        # kw-major dest: dst[c, kw*128 + kh]
                        dvv = dstT[:].rearrange("p (kw kh) -> p kw kh", kh=128)
                        if n == 6:
                            srcv = ps[:].rearrange("p (b x) -> p b x", b=2)[:, :, 0:390]
                            srcv = srcv.rearrange("p b (k s) -> p b k s", k=3)[
                                :, :, :, t * 65:(t + 1) * 65]
                            dv = dvv[:, :, kh0:kh0 + 6].transpose([0, 2, 1]).rearrange(
                                "p (b k) s -> p b k s", b=2)
                        else:
                            nb = min(3, n)
                            srcv = ps[:, 0:nb * 130].rearrange(
                                "p (k s) -> p k s", k=nb)[:, :, t * 65:(t + 1) * 65]
                            dv = dvv[:, :, kh0:kh0 + nb].transpose([0, 2, 1])
                        if eng_scalar:
                            nc.scalar.copy(dv, srcv)
                        else:
                            nc.vector.tensor_copy(dv, srcv)

                if g + 1 < G:
                    for ch in range(0, 16 * NPRE, 16):
                        xr_pre[g + 1].append(emit_xchunk(rep, g + 1, ch))

                NF = KH * KW  # 8320

                # S3: MLP layer 1 + relu -> Hr/Hi bf16 (pr|pi share one 2-bank tile)
                Hr = big.tile([128, NF], BF16, tag="Hx", bufs=2, name=f"Hr_{rep}_{g}")
                Hi = big.tile([128, NF], BF16, tag="Hx", bufs=2, name=f"Hi_{rep}_{g}")
                for off, n in _chunks(NF, 512):
                    ps = psp.tile(PS_SHAPE, F32, tag="ps", bufs=4, name=f"s3p_{rep}_{g}_{off}")
                    pr, pi = ps[:, 0:512], ps[:, 512:1024]
                    sr_c, si_c = Sr[:, off:off + n], Si[:, off:off + n]
                    nc.tensor.matmul(pr[:, :n], wt["w1r"][:], sr_c, start=True, stop=False)
                    nc.tensor.matmul(pr[:, :n], wt["w1in"][:], si_c, start=False, stop=True)
                    nc.tensor.matmul(pi[:, :n], wt["w1i"][:], sr_c, start=True, stop=False)
                    nc.tensor.matmul(pi[:, :n], wt["w1r"][:], si_c, start=False, stop=True)
                    nc.vector.tensor_scalar(Hr[:, off:off + n], pr[:, :n],
                                            wt["b1r"][:], 0.0, ALU.add, ALU.max)
                    nc.scalar.activation(Hi[:, off:off + n], pi[:, :n], AF.Relu,
                                         bias=wt["b1i"][:])

                # S4 + gate: layer 2, bias, G = O * S (complex) -> Gr/Gi bf16
                Gri = big.tile([128, 2 * NF], BF16, tag="Gri", bufs=1, name=f"Gri_{rep}_{g}")
                Gr = Gri[:, 0:NF]
                Gi = Gri[:, NF:2 * NF]
                BLK = 1024
                orr = oi = None
                for off, n in _chunks(NF, 512):
                    if off % BLK == 0:
                        b0 = off
                        bn = min(BLK, NF - b0)
                        orr = work.tile([128, BLK], BF16, tag="orr", bufs=2,
                                        name=f"or_{rep}_{g}_{off}")
                        oi = work.tile([128, BLK], BF16, tag="oib", bufs=2,
                                       name=f"oi_{rep}_{g}_{off}")
                    ps = psp.tile(PS_SHAPE, F32, tag="ps", bufs=4, name=f"s4p_{rep}_{g}_{off}")
                    por, poi = ps[:, 0:512], ps[:, 512:1024]
                    hr_c, hi_c = Hr[:, off:off + n], Hi[:, off:off + n]
                    nc.tensor.matmul(por[:, :n], wt["w2r"][:], hr_c, start=True, stop=False)
                    nc.tensor.matmul(por[:, :n], wt["w2in"][:], hi_c, start=False, stop=True)
                    nc.tensor.matmul(poi[:, :n], wt["w2i"][:], hr_c, start=True, stop=False)
                    nc.tensor.matmul(poi[:, :n], wt["w2r"][:], hi_c, start=False, stop=True)
                    lo = off - b0
                    nc.scalar.activation(orr[:, lo:lo + n], por[:, :n], AF.Identity,
                                         bias=wt["b2r"][:])
                    nc.scalar.activation(oi[:, lo:lo + n], poi[:, :n], AF.Identity,
                                         bias=wt["b2i"][:])
                    if off + n == b0 + bn:  # block complete -> gate it
                        sr_b, si_b = Sr[:, b0:b0 + bn], Si[:, b0:b0 + bn]
                        t2 = work.tile([128, BLK], BF16, tag="tt", bufs=2,
                                       name=f"t2_{rep}_{g}_{b0}")
                        t4 = work.tile([128, BLK], BF16, tag="tt", bufs=2,
                                       name=f"t4_{rep}_{g}_{b0}")
                        nc.vector.tensor_mul(Gr[:, b0:b0 + bn], orr[:, :bn], sr_b)
                        nc.vector.tensor_mul(Gi[:, b0:b0 + bn], orr[:, :bn], si_b)
                        nc.gpsimd.tensor_mul(t2[:, :bn], oi[:, :bn], si_b)
                        nc.vector.tensor_mul(t4[:, :bn], oi[:, :bn], sr_b)
                        nc.vector.tensor_sub(Gr[:, b0:b0 + bn], Gr[:, b0:b0 + bn], t2[:, :bn])
                        nc.vector.tensor_add(Gi[:, b0:b0 + bn], Gi[:, b0:b0 + bn], t4[:, :bn])

                # S5: PE-transpose G -> GTr/GTi [kh, kw*128 + c] bf16 (16 per tile)
                GTr = big.tile([128, KW * 128], BF16, tag="Hx", bufs=2, name=f"GTr_{rep}_{g}")
                GTi = big.tile([128, KW * 128], BF16, tag="Hx", bufs=2, name=f"GTi_{rep}_{g}")
                Grv = Gr.rearrange("p (k s) -> p k s", s=128)
                Giv = Gi.rearrange("p (k s) -> p k s", s=128)
                for ti, (kw0, n) in enumerate(_chunks(KW, 16)):
                    for t, (srcv, dstT) in enumerate(((Grv, GTr), (Giv, GTi))):
                        ps = psp.tile(PS_SHAPE, F32, tag="ps", bufs=4,
                                      name=f"s5p_{rep}_{g}_{kw0}_{t}")
                        p16 = ps[:].bitcast(BF16)
                        for j in range(n):
                            nc.tensor.transpose(p16[:, j * 128:(j + 1) * 128],
                                                srcv[:, kw0 + j, :], cbf["IDENT"][:])
                        d = dstT[:, kw0 * 128:(kw0 + n) * 128]
                        if (ti + t) % 2 == 0:
                            nc.scalar.copy(d, p16[:, 0:n * 128])
                        else:
                            nc.vector.tensor_copy(d, p16[:, 0:n * 128])

                # S6+S7, software-pipelined: emit S7 for the previous quad
                GTrv = GTr[:].rearrange("p (k c) -> p k c", c=128)
                GTiv = GTi[:].rearrange("p (k c) -> p k c", c=128)

                def emit_s7(cq, vts):
                    psy = psp.tile(PS_SHAPE, F32, tag="ps", bufs=4,
                                   name=f"s7p_{rep}_{g}_{cq}")
                    for j in range(8):
                        vt = vts[j // 4]
                        h0 = (j % 4) * 256
                        vr = vt[:, h0:h0 + 128]
                        vi = vt[:, h0 + 128:h0 + 256]
                        o = psy[:, j * 128:(j + 1) * 128]
                        nc.tensor.matmul(o, vr, cbf["ICr"][:], start=True, stop=False)
                        nc.tensor.matmul(o, vi, cbf["ICi"][:], start=False, stop=True)
                    ysb = work.tile([128, 1024], F32, tag="y", bufs=2,
                                    name=f"y_{rep}_{g}_{cq}")
                    nc.scalar.copy(ysb[:], psy[:])
                    nc.scalar.dma_start(
                        out_ext[c0 + 8 * cq:c0 + 8 * cq + 8].transpose([1, 0, 2]),
                        ysb[:].rearrange("p (c w) -> p c w", w=128))

                prev = None
                for cq in range(D // 8):
                    vts = []
                    for j2 in range(2):
                        ps = psp.tile(PS_SHAPE, F32, tag="ps", bufs=4,
                                      name=f"s6p_{rep}_{g}_{cq}_{j2}")
                        for j in range(4):
                            c = 8 * cq + 4 * j2 + j
                            o = ps[0:65, j * 256:(j + 1) * 256]
                            nc.tensor.matmul(o, GTrv[:, :, c], cbf["IH_A"][:],
                                             start=True, stop=False)
                            nc.tensor.matmul(o, GTiv[:, :, c], cbf["IH_B"][:],
                                             start=False, stop=True)
                        vt = work.tile([65, 1024], BF16, tag="V", bufs=4,
                                       name=f"v_{rep}_{g}_{cq}_{j2}")
                        if (cq + j2) % 2 == 0:
                            nc.scalar.copy(vt[:], ps[0:65, :])
                        else:
                            nc.vector.tensor_copy(vt[:], ps[0:65, :])
                        vts.append(vt)
                    if prev is not None:
                        emit_s7(cq - 1, prev)
                    prev = vts
                emit_s7(D // 8 - 1, prev)

    nc.compile()
    return nc


_NC = None


def _get_nc():
    global _NC
    if _NC is None:
        _NC = build_nc()
    return _NC


def kernel(x, w1, w2, b1, b2, trace=False):
    nc = _get_nc()
    x = np.ascontiguousarray(x, dtype=np.float32)
    ins = {
        "w1": np.ascontiguousarray(w1, dtype=np.float32),
        "w2": np.ascontiguousarray(w2, dtype=np.float32),
        "b1": np.ascontiguousarray(b1, dtype=np.float32),
        "b2": np.ascontiguousarray(b2, dtype=np.float32),
    }
    in_maps = [dict(ins, x=x[i]) for i in range(NCORES)]
    res = run_bass_kernel_spmd(nc, in_maps, list(range(NCORES)), trace=trace)
    out = np.stack([np.asarray(r["out"], dtype=np.float32) for r in res.results])
    if trace:
        return out, res
    return out


# revision 36
# speedup vs baseline: 1.0002x; 1.0002x over previous
"""AFNO-style spectral MLP (nn_AFMM_57166014709855) as a Bass/Tile kernel on 8 TRN2 cores.

Math: y = irfft2( (cMLP(rfft2(x)) * rfft2(x)) ) with a grouped complex MLP in
the frequency domain. All FFTs are implemented as DFT matmuls on the tensor
engine. Data-parallel over batch: core b handles x[b] (512,128,128); weights
replicated; no collectives.

Per-core dataflow for each group g (d=128 channels):
  S1 (contract h): per ch:  psum[w,256] = x_c^T @ [FHr|FHi]        (f32r)
  S2 (contract w): per kh:  psum[c,130] = Yr@[Fwr|Fwi] + Yi@[-Fwi|Fwr]  (bf16)
  S3 (contract c): chunks:  H = relu(W1 (x) S + b1)                 (bf16)
  S4 (contract c): chunks:  O = W2 (x) H + b2; gate G = O * S       (bf16)
  S5: PE-transpose G -> GT [kh, (kw,c)]                             (bf16)
  S6 (contract kh): per ch: psum[kw,256] = GTr@[IHr|IHi] + GTi@[-IHi|IHr]
  S7 (contract kw): per ch: y_c[h,w] = Vr^T@ICr + Vi^T@ICi
"""

import numpy as np

import concourse.bass as bass
import concourse.tile as tile
import concourse.mybir as mybir
from concourse import bacc
from concourse.bass_utils import run_bass_kernel_spmd

F32 = mybir.dt.float32
F32R = mybir.dt.float32r
BF16 = mybir.dt.bfloat16
AF = mybir.ActivationFunctionType
ALU = mybir.AluOpType

B, C, H, W = 8, 512, 128, 128
G, D, KH, KW = 4, 128, 128, 65
NCORES = 8


def _consts():
    s = 1.0 / np.sqrt(128.0)
    k = np.arange(KW)
    n = np.arange(128)
    th_w = np.outer(n, k) * (2 * np.pi / W)          # [w, kw]
    Fwr = np.cos(th_w) * s
    Fwi = -np.sin(th_w) * s
    th_h = np.outer(n, n) * (2 * np.pi / H)          # [h, kh] (symmetric)
    FHr_T = np.cos(th_h) * s
    FHi_T = -np.sin(th_h) * s
    IHr_T = np.cos(th_h) * s
    IHi_T = np.sin(th_h) * s
    a = np.full(KW, 2.0)
    a[0] = 1.0
    a[64] = 1.0
    th_c = np.outer(k, n) * (2 * np.pi / W)          # [kw, w]
    ICr = a[:, None] * np.cos(th_c) * s
    ICi = -a[:, None] * np.sin(th_c) * s
    f32 = lambda x: np.ascontiguousarray(x, dtype=np.float32)
    return {
        "FHRI": f32(np.concatenate([FHr_T[:, 0:65], FHi_T[:, 0:65]], axis=1)),  # [128, 130]
        "FW_A": f32(np.concatenate([Fwr, Fwi], axis=1)),       # [128, 130]
        "FW_B": f32(np.concatenate([-Fwi, Fwr], axis=1)),      # [128, 130]
        "FW_C": f32(np.concatenate([Fwi, -Fwr], axis=1)),      # [128, 130]
        "IH_A": f32(np.concatenate([IHr_T, IHi_T], axis=1)),   # [128, 256]
        "IH_B": f32(np.concatenate([-IHi_T, IHr_T], axis=1)),  # [128, 256]
        "ICr": f32(ICr),                                       # [65, 128]
        "ICi": f32(ICi),                                       # [65, 128]
        "IDENT": f32(np.eye(128)),                             # [128, 128]
    }


def _chunks(total, step):
    off = 0
    while off < total:
        n = min(step, total - off)
        yield off, n
        off += n


def build_nc():
    nc = bacc.Bacc(None, target_bir_lowering=False, debug=False)

    x_ext = nc.declare_dram_parameter("x", [C, H, W], F32, isOutput=False)
    w1_ext = nc.declare_dram_parameter("w1", [2, G, D, D], F32, isOutput=False)
    w2_ext = nc.declare_dram_parameter("w2", [2, G, D, D], F32, isOutput=False)
    b1_ext = nc.declare_dram_parameter("b1", [2, G, D], F32, isOutput=False)
    b2_ext = nc.declare_dram_parameter("b2", [2, G, D], F32, isOutput=False)
    out_ext = nc.declare_dram_parameter("out", [C, H, W], F32, isOutput=True)

    cdat = _consts()
    cdram = {k: nc.inline_tensor(v, f"c_{k}") for k, v in cdat.items()}

    with tile.TileContext(nc) as tc:
        with (
            tc.tile_pool(name="cst", bufs=1) as cst,
            tc.tile_pool(name="big", bufs=1) as big,
            tc.tile_pool(name="work", bufs=1) as work,
            tc.tile_pool(name="psum", bufs=8, space="PSUM") as psp,
        ):
            # ---- constants: DMA fp32 staging, convert once on gpsimd ----
            def staged(k):
                t = cst.tile([128, 256], F32, tag="stg", bufs=2, name=f"stg_{k}")
                v = t[:, 0:cdat[k].shape[1]][0:cdat[k].shape[0], :]
                nc.scalar.dma_start(v, cdram[k][:])
                return v
            FHRI = cst.tile([128, 130], BF16, name="FHRI")
            nc.vector.tensor_copy(FHRI[:], staged("FHRI"))
            cbf = {}
            for k in ["FW_A", "FW_B", "FW_C", "IH_A", "IH_B", "ICr", "ICi", "IDENT"]:
                t = cst.tile(list(cdat[k].shape), BF16, name=f"bf_{k}")
                nc.vector.tensor_copy(t[:], staged(k))
                cbf[k] = t

            # ---- weights/biases for all 4 groups ----
            wts = []  # per group dict
            for g in range(G):
                d = {}
                for nm, ext, comp in [
                    ("w1r", w1_ext, 0), ("w1i", w1_ext, 1),
                    ("w2r", w2_ext, 0), ("w2i", w2_ext, 1),
                ]:
                    wstg = cst.tile([128, 128], F32, tag="wstg", bufs=2,
                                    name=f"wstg_{g}_{nm}")
                    nc.scalar.dma_start(wstg[:], ext[comp, g])
                    wbf = cst.tile([128, 128], BF16, name=f"w_{g}_{nm}")
                    nc.vector.tensor_copy(wbf[:], wstg[:])
                    d[nm] = wbf
                    if nm in ("w1i", "w2i"):
                        wneg = cst.tile([128, 128], BF16, name=f"w_{g}_{nm}n")
                        nc.vector.tensor_scalar_mul(wneg[:], wstg[:], -1.0)
                        d[nm + "n"] = wneg
                for nm, ext, comp in [
                    ("b1r", b1_ext, 0), ("b1i", b1_ext, 1),
                    ("b2r", b2_ext, 0), ("b2i", b2_ext, 1),
                ]:
                    bt = cst.tile([128, 1], F32, name=f"b_{g}_{nm}")
                    nc.scalar.dma_start(bt[:], ext[comp, g].rearrange("(p one) -> p one", one=1))
                    d[nm] = bt
                wts.append(d)

            # ---- per-group pipeline ----
            PS_SHAPE = [128, 1024]  # 2 PSUM banks
            NPRE = 16

            def emit_xchunk(rep, g, ch):
                c0 = g * D
                xf = work.tile([128, 1024], F32, tag="xf", bufs=3,
                               name=f"xf_{rep}_{g}_{ch}", uniquify=False)
                eng = nc.sync if (ch // 8) % 2 == 0 else nc.scalar
                eng.dma_start(
                    xf[:].rearrange("p (c w) -> p c w", w=128),
                    x_ext[c0 + ch:c0 + ch + 8].transpose([1, 0, 2]))
                xr = work.tile([128, 1024], BF16, tag="xr", bufs=4,
                               name=f"xr_{rep}_{g}_{ch}", uniquify=False)
                nc.gpsimd.tensor_copy(xr[:, 0:1024], xf[:, 0:1024])
                nc.vector.tensor_copy(xr[:, 1024:2048], xf[:, 1024:2048])
                return xr

            for rep in range(repeat):
              xr_pre = [[] for _ in range(G)]
              for ch in range(0, 16 * NPRE, 16):
                  xr_pre[0].append(emit_xchunk(rep, 0, ch, wide=(rep == 0)))
              for g in range(G):
                wt = wts[g]
                c0 = g * D

                # x load + convert to bf16, 8 channels per chunk
                # (first NPRE chunks of each group are emitted early, after the
                #  previous group's S2, so converts overlap the gate/MLP era)
                xr_tiles = xr_pre[g]
                for ch in range(16 * len(xr_tiles), D, 16):
                    xr_tiles.append(emit_xchunk(rep, g, ch, wide=(rep == 0 and g == 0)))

                # S1: forward H-FFT (hermitian half: kh 0..64)
                # -> Y [w, c*130 + comp*65 + kh] bf16; 6 channels per psum tile
                Y = big.tile([128, D * 130], BF16, tag="Y", bufs=1, name=f"Y_{rep}_{g}")
                for ti, (ch0, n) in enumerate(_chunks(D, 6)):
                    ps = psp.tile(PS_SHAPE, F32, tag="ps", bufs=4, name=f"s1p_{rep}_{g}_{ch0}")
                    for j in range(n):
                        c = ch0 + j
                        off = (j % 3) * 130 + (j // 3) * 512
                        lhsT = xr_tiles[c // 16][:, (c % 16) * 128:(c % 16) * 128 + 128]
                        nc.tensor.matmul(ps[:, off:off + 130], lhsT, FHRI[:],
                                         start=True, stop=True)
                    if n == 6:
                        srcv = ps[:].rearrange("p (b x) -> p b x", b=2)[:, :, 0:390]
                        dv = Y[:, ch0 * 130:(ch0 + 6) * 130].rearrange(
                            "p (b x) -> p b x", b=2)
                    else:
                        srcv = ps[:, 0:n * 130]
                        dv = Y[:, ch0 * 130:(ch0 + n) * 130]
                    if ti % 2 == 0:
                        nc.scalar.copy(dv, srcv)
                    else:
                        nc.vector.tensor_copy(dv, srcv)

                # S2: forward W-FFT -> Sr/Si [c, kh*65 + kw] bf16; 6 kh per tile
                Sr = big.tile([128, KH * KW], BF16, tag="Sr", bufs=1, name=f"Sr_{rep}_{g}")
                Si = big.tile([128, KH * KW], BF16, tag="Si", bufs=1, name=f"Si_{rep}_{g}")
                Yv = Y[:].rearrange("p (c t k) -> p c t k", t=2, k=65)
                for ti, (kh0, n) in enumerate(_chunks(KH, 6)):
                    ps = psp.tile(PS_SHAPE, F32, tag="ps", bufs=4, name=f"s2p_{rep}_{g}_{kh0}")
                    for j in range(n):
                        kh = kh0 + j
                        idx = kh if kh <= 64 else 128 - kh
                        rhsB = cbf["FW_B"] if kh <= 64 else cbf["FW_C"]
                        off = j * 130 if j < 3 else 512 + (j - 3) * 130
                        o = ps[:, off:off + 130]
                        nc.tensor.matmul(o, Yv[:, :, 0, idx], cbf["FW_A"][:],
                                         start=True, stop=False)
                        nc.tensor.matmul(o, Yv[:, :, 1, idx], rhsB[:],
                                         start=False, stop=True)
                    for t, dstT in ((0, Sr), (1, Si)):
                        eng_scalar = (ti + t) % 2 == 0
                        # kw-major dest: dst[c, kw*128 + kh]
                        dvv = dstT[:].rearrange("p (kw kh) -> p kw kh", kh=128)
                        if n == 6:
                            srcv = ps[:].rearrange("p (b x) -> p b x", b=2)[:, :, 0:390]
                            srcv = srcv.rearrange("p b (k s) -> p b k s", k=3)[
                                :, :, :, t * 65:(t + 1) * 65]
                            dv = dvv[:, :, kh0:kh0 + 6].transpose([0, 2, 1]).rearrange(
                                "p (b k) s -> p b k s", b=2)
                        else:
                            nb = min(3, n)
                            srcv = ps[:, 0:nb * 130].rearrange(
                                "p (k s) -> p k s", k=nb)[:, :, t * 65:(t + 1) * 65]
                            dv = dvv[:, :, kh0:kh0 + nb].transpose([0, 2, 1])
                        if eng_scalar:
                            nc.scalar.copy(dv, srcv)
                        else:
                            nc.vector.tensor_copy(dv, srcv)

                if g + 1 < G:
                    for ch in range(0, 16 * NPRE, 16):
                        xr_pre[g + 1].append(emit_xchunk(rep, g + 1, ch))

                NF = KH * KW  # 8320

                # S3: MLP layer 1 + relu -> Hr/Hi bf16 (pr|pi share one 2-bank tile)
                Hr = big.tile([128, NF], BF16, tag="Hx", bufs=2, name=f"Hr_{rep}_{g}")
                Hi = big.tile([128, NF], BF16, tag="Hx", bufs=2, name=f"Hi_{rep}_{g}")
                for off, n in _chunks(NF, 512):
                    ps = psp.tile(PS_SHAPE, F32, tag="ps", bufs=4, name=f"s3p_{rep}_{g}_{off}")
                    pr, pi = ps[:, 0:512], ps[:, 512:1024]
                    sr_c, si_c = Sr[:, off:off + n], Si[:, off:off + n]
                    nc.tensor.matmul(pr[:, :n], wt["w1r"][:], sr_c, start=True, stop=False)
                    nc.tensor.matmul(pr[:, :n], wt["w1in"][:], si_c, start=False, stop=True)
                    nc.tensor.matmul(pi[:, :n], wt["w1i"][:], sr_c, start=True, stop=False)
                    nc.tensor.matmul(pi[:, :n], wt["w1r"][:], si_c, start=False, stop=True)
                    nc.vector.tensor_scalar(Hr[:, off:off + n], pr[:, :n],
                                            wt["b1r"][:], 0.0, ALU.add, ALU.max)
                    nc.scalar.activation(Hi[:, off:off + n], pi[:, :n], AF.Relu,
                                         bias=wt["b1i"][:])

                # S4 + gate: layer 2, bias, G = O * S (complex) -> Gr/Gi bf16
                Gri = big.tile([128, 2 * NF], BF16, tag="Gri", bufs=1, name=f"Gri_{rep}_{g}")
                Gr = Gri[:, 0:NF]
                Gi = Gri[:, NF:2 * NF]
                BLK = 1024
                orr = oi = None
                for off, n in _chunks(NF, 512):
                    if off % BLK == 0:
                        b0 = off
                        bn = min(BLK, NF - b0)
                        orr = work.tile([128, BLK], BF16, tag="orr", bufs=2,
                                        name=f"or_{rep}_{g}_{off}")
                        oi = work.tile([128, BLK], BF16, tag="oib", bufs=2,
                                       name=f"oi_{rep}_{g}_{off}")
                    ps = psp.tile(PS_SHAPE, F32, tag="ps", bufs=4, name=f"s4p_{rep}_{g}_{off}")
                    por, poi = ps[:, 0:512], ps[:, 512:1024]
                    hr_c, hi_c = Hr[:, off:off + n], Hi[:, off:off + n]
                    nc.tensor.matmul(por[:, :n], wt["w2r"][:], hr_c, start=True, stop=False)
                    nc.tensor.matmul(por[:, :n], wt["w2in"][:], hi_c, start=False, stop=True)
                    nc.tensor.matmul(poi[:, :n], wt["w2i"][:], hr_c, start=True, stop=False)
                    nc.tensor.matmul(poi[:, :n], wt["w2r"][:], hi_c, start=False, stop=True)
                    lo = off - b0
                    nc.scalar.activation(orr[:, lo:lo + n], por[:, :n], AF.Identity,
                                         bias=wt["b2r"][:])
                    nc.scalar.activation(oi[:, lo:lo + n], poi[:, :n], AF.Identity,
                                         bias=wt["b2i"][:])
                    if off + n == b0 + bn:  # block complete -> gate it
                        sr_b, si_b = Sr[:, b0:b0 + bn], Si[:, b0:b0 + bn]
                        t2 = work.tile([128, BLK], BF16, tag="tt", bufs=2,
                                       name=f"t2_{rep}_{g}_{b0}")
                        t4 = work.tile([128, BLK], BF16, tag="tt", bufs=2,
                                       name=f"t4_{rep}_{g}_{b0}")
                        nc.vector.tensor_mul(Gr[:, b0:b0 + bn], orr[:, :bn], sr_b)
                        nc.vector.tensor_mul(Gi[:, b0:b0 + bn], orr[:, :bn], si_b)
                        nc.gpsimd.tensor_mul(t2[:, :bn], oi[:, :bn], si_b)
                        nc.vector.tensor_mul(t4[:, :bn], oi[:, :bn], sr_b)
                        nc.vector.tensor_sub(Gr[:, b0:b0 + bn], Gr[:, b0:b0 + bn], t2[:, :bn])
                        nc.vector.tensor_add(Gi[:, b0:b0 + bn], Gi[:, b0:b0 + bn], t4[:, :bn])

                # S5: PE-transpose G -> GTr/GTi [kh, kw*128 + c] bf16 (16 per tile)
                GTr = big.tile([128, KW * 128], BF16, tag="Hx", bufs=2, name=f"GTr_{rep}_{g}")
                GTi = big.tile([128, KW * 128], BF16, tag="Hx", bufs=2, name=f"GTi_{rep}_{g}")
                Grv = Gr.rearrange("p (k s) -> p k s", s=128)
                Giv = Gi.rearrange("p (k s) -> p k s", s=128)
                for ti, (kw0, n) in enumerate(_chunks(KW, 16)):
                    for t, (srcv, dstT) in enumerate(((Grv, GTr), (Giv, GTi))):
                        ps = psp.tile(PS_SHAPE, F32, tag="ps", bufs=4,
                                      name=f"s5p_{rep}_{g}_{kw0}_{t}")
                        p16 = ps[:].bitcast(BF16)
                        for j in range(n):
                            nc.tensor.transpose(p16[:, j * 128:(j + 1) * 128],
                                                srcv[:, kw0 + j, :], cbf["IDENT"][:])
                        d = dstT[:, kw0 * 128:(kw0 + n) * 128]
                        if (ti + t) % 2 == 0:
                            nc.scalar.copy(d, p16[:, 0:n * 128])
                        else:
                            nc.vector.tensor_copy(d, p16[:, 0:n * 128])

                # S6+S7, software-pipelined: emit S7 for the previous quad
                GTrv = GTr[:].rearrange("p (k c) -> p k c", c=128)
                GTiv = GTi[:].rearrange("p (k c) -> p k c", c=128)

                def emit_s7(cq, vts):
                    psy = psp.tile(PS_SHAPE, F32, tag="ps", bufs=4,
                                   name=f"s7p_{rep}_{g}_{cq}")
                    for j in range(8):
                        vt = vts[j // 4]
                        h0 = (j % 4) * 256
                        vr = vt[:, h0:h0 + 128]
                        vi = vt[:, h0 + 128:h0 + 256]
                        o = psy[:, j * 128:(j + 1) * 128]
                        nc.tensor.matmul(o, vr, cbf["ICr"][:], start=True, stop=False)
                        nc.tensor.matmul(o, vi, cbf["ICi"][:], start=False, stop=True)
                    ysb = work.tile([128, 1024], F32, tag="y", bufs=2,
                                    name=f"y_{rep}_{g}_{cq}")
                    if cq % 4 == 3:
                        nc.vector.tensor_copy(ysb[:], psy[:])
                    else:
                        nc.scalar.copy(ysb[:], psy[:])
                    nc.scalar.dma_start(
                        out_ext[c0 + 8 * cq:c0 + 8 * cq + 8].transpose([1, 0, 2]),
                        ysb[:].rearrange("p (c w) -> p c w", w=128))

                prev = None
                for cq in range(D // 8):
                    vts = []
                    for j2 in range(2):
                        ps = psp.tile(PS_SHAPE, F32, tag="ps", bufs=4,
                                      name=f"s6p_{rep}_{g}_{cq}_{j2}")
                        for j in range(4):
                            c = 8 * cq + 4 * j2 + j
                            o = ps[0:65, j * 256:(j + 1) * 256]
                            nc.tensor.matmul(o, GTrv[:, :, c], cbf["IH_A"][:],
                                             start=True, stop=False)
                            nc.tensor.matmul(o, GTiv[:, :, c], cbf["IH_B"][:],
                                             start=False, stop=True)
                        vt = work.tile([65, 1024], BF16, tag="V", bufs=4,
                                       name=f"v_{rep}_{g}_{cq}_{j2}")
                        if (cq + j2) % 2 == 0:
                            nc.scalar.copy(vt[:], ps[0:65, :])
                        else:
                            nc.vector.tensor_copy(vt[:], ps[0:65, :])
                        vts.append(vt)
                    if prev is not None:
                        emit_s7(cq - 1, prev)
                    prev = vts
                emit_s7(D // 8 - 1, prev)

    nc.compile()
    return nc


_NC = None


def _get_nc():
    global _NC
    if _NC is None:
        _NC = build_nc()
    return _NC


def kernel(x, w1, w2, b1, b2, trace=False):
    nc = _get_nc()
    x = np.ascontiguousarray(x, dtype=np.float32)
    ins = {
        "w1": np.ascontiguousarray(w1, dtype=np.float32),
        "w2": np.ascontiguousarray(w2, dtype=np.float32),
        "b1": np.ascontiguousarray(b1, dtype=np.float32),
        "b2": np.ascontiguousarray(b2, dtype=np.float32),
    }
    in_maps = [dict(ins, x=x[i]) for i in range(NCORES)]
    res = run_bass_kernel_spmd(nc, in_maps, list(range(NCORES)), trace=trace)
    out = np.stack([np.asarray(r["out"], dtype=np.float32) for r in res.results])
    if trace:
        return out, res
    return out
